# revision 44
# baseline (speedup 1.0000x reference)
"""Llama4TextExperts MoE kernel for 8 Trainium2 NeuronCores — v4 (bf16).

Expert-parallel: core e handles expert e (tokens pre-sorted per expert).
Per core: x_e (1024,2048) @ gate_up[e] (2048,8192) -> silu(gate)*up ->
@ down[e] (4096,2048) -> out_e (1024,2048).

v2 (bf16): all matmuls bf16 (rel err ~4e-3 vs 2e-2 gate); halves weight
DMA vs fp32r and keeps acted SBUF-resident; LDW dedup post-pass (one
LDWEIGHTS per weight tile, serving both 512-token chunks).

v4 on top of v2 (measured HW findings that motivated it):
- HW probes showed the kernel is matmul-stream-bound: removing all
  LDWEIGHTS + weight DMA (shared-weight probe) did not change exec time,
  so LDW is fully hidden and DMA is not on the critical path. Sustained
  per-MM rate ~270 ns at N=512 (PE ~1.95-2.0 GHz power state); body time
  via repeat-8/repeat-4 NEFFs = ~830 us vs 807 us pure-MM -> ~22 us of
  non-MM slack, i.e. near the sustained-power roofline for bf16.
- bf16 output (+ host upcast) and bf16 silu temp: halves output DMA and
  silu SBUF traffic; rel err 6.3e-3, still 3x under the gate.
- Startup/transition trims (v6, CoreSim-gap-driven): the startup window
  is HBM-BW-saturated, so issue order is consumption order — x[0], the
  first two kk-slices of the g=0 weights (first matmul at ~0.6us),
  x[1..3], weight remainder, x[4..15]; the 2 MB w2 phase-2 prefetch is
  issued mid-phase-1 (g==6) where the DMA queue has slack. Last h's
  output is copy+DMA'd per 512-chunk so the final DMA overlaps the
  second copy. Model: 677.5us (v2) -> 664.2us, PE gaps 21us -> ~9us.
- Sem-update coalescing post-pass: 3072 per-MM +1 posts -> ~160 waited
  checkpoints (accumulated base rides an in-order PE NoOp; the +1 at
  each checkpoint MM still fires at its completion, so no wait can be
  satisfied early). Measured neutral on HW but reduces NX sem traffic.
- fuse=True (wide [128,1024] PSUM tiles, single wide act/mul) compiles
  but hangs the device: Act/DVE APs must not cross a PSUM bank boundary,
  and DVE TensorTensor may read at most one PSUM operand. Kept off.
"""

import numpy as np

NUM_EXPERTS = 8
HIDDEN = 2048
INTER = 4096
TOKENS = 8192
T = TOKENS // NUM_EXPERTS  # 1024 tokens per expert/core
TK = HIDDEN // 128  # 16 contraction tiles in phase 1
TI = INTER // 128  # 32 feature tiles of gate/up; contraction tiles in phase 2
TH = HIDDEN // 128  # 16 output feature tiles
NT = T // 512  # 2 token chunks of 512


def _split_waits(nc, max_waits=1):
    """The walrus build in this environment rejects instructions carrying
    more than one sync wait. Move excess SyncWaits onto preceding NoOps
    on the same engine."""
    import concourse.mybir as mybir

    for fn in nc.m.functions:
        for blk in fn.blocks:
            new_insts = []
            for inst in blk.instructions:
                si = inst.sync_info
                if si is not None and len(si.on_wait) > max_waits:
                    waits = list(si.on_wait)
                    excess, keep = waits[:-max_waits], waits[-max_waits:]
                    for i in range(0, len(excess), max_waits):
                        chunk = excess[i : i + max_waits]
                        new_insts.append(
                            mybir.InstNoOp(
                                name=f"{inst.name}-waitsplit-{i}",
                                ins=[],
                                outs=[],
                                engine=inst.engine,
                                sync_info=mybir.SyncInfo(
                                    on_wait=list(chunk), on_update=[]
                                ),
                            )
                        )
                    si.on_wait = keep
                new_insts.append(inst)
            blk.instructions = new_insts


def _dedup_ldweights(nc):
    """Remove InstLdweights whose weights AP equals the one already loaded
    (Tile emits 1:1 LDW:MM). Waits/updates of a removed LDW attach to the
    next instruction: both then take effect later than the original LDW
    would have, which is always safe."""
    import concourse.mybir as mybir

    removed = 0
    for fn in nc.m.functions:
        for blk in fn.blocks:
            loaded = None
            pending = []
            final = []
            for inst in blk.instructions:
                if isinstance(inst, mybir.InstLdweights):
                    key = repr(inst.ins[0])
                    if loaded == key and inst.perf_mode is None:
                        si = inst.sync_info
                        if si is not None:
                            pending.append(
                                (list(si.on_wait), list(si.on_update))
                            )
                        removed += 1
                        continue
                    loaded = key
                elif isinstance(inst, mybir.InstMatmult) and inst.is_transpose:
                    loaded = None
                if pending:
                    si = inst.sync_info
                    if si is None:
                        inst.sync_info = mybir.SyncInfo(on_wait=[], on_update=[])
                        si = inst.sync_info
                    for pw, pu in pending:
                        si.on_wait = list(si.on_wait) + pw
                        si.on_update = list(si.on_update) + pu
                    pending = []
                final.append(inst)
            blk.instructions = final
    return removed


def _coalesce_pe_updates(nc):
    """Each matmul posts +1 to the PE counting semaphore, but only ~160
    counts are ever waited on (group completions). Remove the per-MM posts
    and emit one accumulated sem-add at each waited count instead. Waits are
    sem-ge-imm (monotonic), so posting the same totals later-but-batched
    preserves all orderings; counts at every waited threshold are exact."""
    import concourse.mybir as mybir

    # identify the PE counting sem: the one inc'd by matmuls
    from collections import Counter

    upd_count = Counter()
    for fn in nc.m.functions:
        for blk in fn.blocks:
            for inst in blk.instructions:
                if isinstance(inst, mybir.InstMatmult) and inst.sync_info:
                    for u in inst.sync_info.on_update:
                        if u.update_mode == "sem-inc" and u.update_value == 1:
                            upd_count[u.id] += 1
    if not upd_count:
        return 0
    sem_id, n = upd_count.most_common(1)[0]
    if n < 100:
        return 0

    thresholds = set()
    for fn in nc.m.functions:
        for blk in fn.blocks:
            for inst in blk.instructions:
                if inst.sync_info:
                    for w in inst.sync_info.on_wait:
                        if w.id == sem_id:
                            thresholds.add(w.wait_value)

    def catchup_noop(name, pending):
        # Generic PE-queue instruction carrying the accumulated base add.
        # NoOps execute in order (only LDWEIGHTS gets pulled ahead), and no
        # wait threshold falls inside the covered range, so posting the base
        # early-at-dispatch is safe: waited counts are only reached by the
        # +1 incs that fire at the checkpoint MMs' own completion.
        return mybir.InstNoOp(
            name=name,
            ins=[],
            outs=[],
            engine=mybir.EngineType.PE,
            sync_info=mybir.SyncInfo(
                on_wait=[],
                on_update=[
                    mybir.SyncUpdate(
                        sync_type="semaphore",
                        id=sem_id,
                        update_mode="sem-add-imm",
                        update_value=pending,
                    )
                ],
            ),
        )

    removed = 0
    for fn in nc.m.functions:
        for blk in fn.blocks:
            count = 0
            pending = 0
            new_insts = []
            for inst in blk.instructions:
                si = inst.sync_info
                ours = (
                    [u for u in si.on_update if u.id == sem_id] if si else []
                )
                if (
                    ours
                    and isinstance(inst, mybir.InstMatmult)
                    and len(ours) == 1
                    and ours[0].update_mode == "sem-inc"
                    and ours[0].update_value == 1
                ):
                    count += 1
                    pending += 1
                    if count in thresholds:
                        if pending > 1:
                            new_insts.append(
                                catchup_noop(f"{inst.name}-semcatch", pending - 1)
                            )
                            removed += pending - 1
                        pending = 0
                        # keep the +1 inc on this MM
                    else:
                        si.on_update = [
                            x for x in si.on_update if x is not ours[0]
                        ]
                elif ours:
                    # foreign update on this sem: post pending base first
                    if pending:
                        new_insts.append(
                            catchup_noop(f"{inst.name}-semflush", pending)
                        )
                        pending = 0
                new_insts.append(inst)
            if pending:
                new_insts.append(catchup_noop(f"{blk.name}-semtail", pending))
            blk.instructions = new_insts
    return removed


def build_bass(
    repeat=1, postpasses=True, probe=None, dedup=True, coalesce=True, fuse=False
):
    # fuse=True (wide [128,1024] PSUM tiles + single wide act/mul/copy per
    # block) compiles but hangs the device — Act/DVE reads crossing a PSUM
    # bank boundary appear to be illegal on HW. Keep off.
    import contextlib

    import concourse.bass as bass
    import concourse.mybir as mybir
    import concourse.tile as tile

    F32 = mybir.dt.float32
    BF16 = mybir.dt.bfloat16
    Silu = mybir.ActivationFunctionType.Silu

    nc = bass.Bass()
    xT = nc.declare_dram_parameter("xT", [HIDDEN, T], BF16, isOutput=False)
    # host-reordered tile-major: w1[g, p, kk, f] = gate_up[kk*128+p, g*128+f]
    # (g 0..31 = gate blocks, 32..63 = up blocks); w2[h, p, ii, f] = down[ii*128+p, h*128+f]
    w1 = nc.declare_dram_parameter("w1", [2 * TI, 128, TK, 128], BF16, isOutput=False)
    w2 = nc.declare_dram_parameter("w2", [TH, 128, TI, 128], BF16, isOutput=False)
    # bf16 output (host upcasts): halves the output DMA + copy traffic; adds
    # <=0.4% rounding on top of ~0.44% — still well under the 2e-2 gate
    outT = nc.declare_dram_parameter("outT", [HIDDEN, T], BF16, isOutput=True)

    xT_t = xT.rearrange("(kk p) t -> kk p t", p=128)
    outT_t = outT.rearrange("(hh p) t -> hh p t", p=128)

    with tile.TileContext(nc) as tc:
        rep = tc.For_i(0, repeat, 1) if repeat > 1 else contextlib.nullcontext()
        with rep:
            with tc.tile_pool(name="xres", bufs=1) as xp, \
                 tc.tile_pool(name="w1s", bufs=3) as w1p, \
                 tc.tile_pool(name="tmp", bufs=8) as tmpp, \
                 tc.tile_pool(name="acted", bufs=1) as actp, \
                 tc.tile_pool(name="w2s", bufs=3) as w2p, \
                 tc.tile_pool(name="outs", bufs=4) as outp, \
                 tc.tile_pool(name="ps", bufs=2, space="PSUM") as psp:
                xts = [
                    xp.tile([128, T], BF16, tag=f"x{k}", name=f"xres{k}")
                    for k in range(TK)
                ]
                # DMA issue order tuned for startup latency: first 4 x tiles,
                # then the g=0 gate/up weights, then the remaining x. The
                # startup window is HBM-BW-saturated, so the 2 MB phase-2 w2
                # prefetch is issued mid-phase-1 (g==6) instead — it only has
                # to land ~650us later.
                wg0 = wu0 = w2t0 = w2t1 = None
                nc.sync.dma_start(out=xts[0], in_=xT_t[0])
                if probe not in ("noldw", "mmonly"):
                    wg0 = w1p.tile([128, TK, 128], BF16, tag="wg")
                    wu0 = w1p.tile([128, TK, 128], BF16, tag="wu")
                    nc.sync.dma_start(out=wg0[:, 0:2, :], in_=w1[0, :, 0:2, :])
                    nc.sync.dma_start(out=wu0[:, 0:2, :], in_=w1[TI, :, 0:2, :])
                for k in range(1, 4):
                    nc.sync.dma_start(out=xts[k], in_=xT_t[k])
                if probe not in ("noldw", "mmonly"):
                    nc.sync.dma_start(out=wg0[:, 2:, :], in_=w1[0, :, 2:, :])
                    nc.sync.dma_start(out=wu0[:, 2:, :], in_=w1[TI, :, 2:, :])
                    w2t0 = w2p.tile([128, TI, 128], BF16, tag="w2")
                    w2t1 = w2p.tile([128, TI, 128], BF16, tag="w2")
                for k in range(4, TK):
                    nc.sync.dma_start(out=xts[k], in_=xT_t[k])

                actts = (
                    [
                        actp.tile([128, T], BF16, tag=f"a{i}", name=f"acted{i}")
                        for i in range(TI)
                    ]
                    if probe not in ("noact", "mmonly")
                    else None
                )

                # probe="mmonly": flat stream of 3072 N=512 matmuls, shared
                # weights, 2 psum banks, two long accumulation groups — no
                # consumers, no bank cycling. Isolates the raw MM issue rate.
                if probe == "mmonly":
                    w_sh = w1p.tile([128, TK, 128], BF16, tag="wg")
                    nc.sync.dma_start(out=w_sh, in_=w1[0])
                    ps = [
                        psp.tile([128, 512], F32, tag=f"pg{t}", name=f"psmm{t}")
                        for t in range(2)
                    ]
                    NMM = 3072
                    for i in range(NMM):
                        b = i % 2
                        nc.tensor.matmul(
                            ps[b],
                            w_sh[:, 0, :],
                            xts[i % TK][:, (i // TK % NT) * 512:(i // TK % NT) * 512 + 512],
                            start=(i < 2),
                            stop=(i >= NMM - 2),
                        )
                    ot = outp.tile([128, T], BF16, tag="ot")
                    for t in range(NT):
                        nc.vector.tensor_copy(ot[:, t * 512:(t + 1) * 512], ps[t])
                    nc.sync.dma_start(out=outT_t[0], in_=ot)

                # probe="noldw": single shared weight tile for every matmul so
                # the dedup pass strips all but ~1 LDWEIGHTS — isolates the
                # serialized LDW cost on HW (output is garbage; timing only).
                if probe == "noldw":
                    wg_sh = w1p.tile([128, TK, 128], BF16, tag="wg")
                    wu_sh = wg_sh
                    nc.sync.dma_start(out=wg_sh, in_=w1[0])

                # ---- Phase 1: gate/up projection + silu(gate)*up -> acted ----
                for g in range(TI if probe != "mmonly" else 0):
                    if probe == "noldw":
                        wg, wu = wg_sh, wu_sh
                    elif g == 0:
                        wg, wu = wg0, wu0
                    else:
                        wg = w1p.tile([128, TK, 128], BF16, tag="wg")
                        wu = w1p.tile([128, TK, 128], BF16, tag="wu")
                        nc.sync.dma_start(out=wg, in_=w1[g])
                        nc.sync.dma_start(out=wu, in_=w1[TI + g])
                    if g == 6 and probe not in ("noldw", "mmonly"):
                        nc.sync.dma_start(out=w2t0, in_=w2[0])
                        nc.sync.dma_start(out=w2t1, in_=w2[1])
                    if fuse:
                        # [128,1024] psum tiles span 2 banks; each matmul
                        # still writes a single bank. silu is computed
                        # in-place in psum (no SBUF temp), then one wide DVE
                        # mul produces the bf16 acted tile.
                        pgb = psp.tile([128, T], F32, tag="pg", name=f"psg{g}")
                        pub = psp.tile([128, T], F32, tag="pu", name=f"psu{g}")
                        pg = [pgb[:, t * 512:(t + 1) * 512] for t in range(NT)]
                        pu = [pub[:, t * 512:(t + 1) * 512] for t in range(NT)]
                    else:
                        pg = [
                            psp.tile([128, 512], F32, tag=f"pg{t}", name=f"psg{g}_{t}")
                            for t in range(NT)
                        ]
                        pu = [
                            psp.tile([128, 512], F32, tag=f"pu{t}", name=f"psu{g}_{t}")
                            for t in range(NT)
                        ]
                    for kk in range(TK):
                        st, sp = kk == 0, kk == TK - 1
                        wk = 0 if probe == "noldw" else kk
                        for t in range(NT):
                            ts = slice(t * 512, (t + 1) * 512)
                            nc.tensor.matmul(
                                pg[t], wg[:, wk, :], xts[kk][:, ts], start=st, stop=sp
                            )
                        for t in range(NT):
                            ts = slice(t * 512, (t + 1) * 512)
                            nc.tensor.matmul(
                                pu[t], wu[:, wk, :], xts[kk][:, ts], start=st, stop=sp
                            )
                    if probe != "noact":
                        if fuse:
                            # DVE can read at most one PSUM operand: silu
                            # goes to an SBUF bf16 temp, one wide instr each
                            sgw = tmpp.tile([128, T], BF16, tag="sg")
                            nc.scalar.activation(sgw, pgb, Silu)
                            nc.vector.tensor_mul(actts[g], sgw, pub)
                        else:
                            for t in range(NT):
                                ts = slice(t * 512, (t + 1) * 512)
                                sg = tmpp.tile([128, 512], BF16, tag="sg")
                                nc.scalar.activation(sg, pg[t], Silu)
                                nc.vector.tensor_mul(actts[g][:, ts], sg, pu[t])

                # ---- Phase 2: down projection -> outT ----
                if probe == "noldw":
                    w2_sh = w2p.tile([128, TI, 128], BF16, tag="w2")
                    nc.sync.dma_start(out=w2_sh, in_=w2[0])
                for h in range(TH if probe != "mmonly" else 0):
                    if probe == "noldw":
                        w2h = w2_sh
                    elif h == 0:
                        w2h = w2t0
                    elif h == 1:
                        w2h = w2t1
                    else:
                        w2h = w2p.tile([128, TI, 128], BF16, tag="w2")
                        nc.sync.dma_start(out=w2h, in_=w2[h])
                    if fuse:
                        pob = psp.tile([128, T], F32, tag="pg", name=f"pso{h}")
                        po = [pob[:, t * 512:(t + 1) * 512] for t in range(NT)]
                    else:
                        po = [
                            psp.tile([128, 512], F32, tag=f"pg{t}", name=f"pso{h}_{t}")
                            for t in range(NT)
                        ]
                    for ii in range(TI):
                        st, sp = ii == 0, ii == TI - 1
                        wi = 0 if probe == "noldw" else ii
                        mov = xts[ii % TK] if probe == "noact" else actts[ii]
                        for t in range(NT):
                            ts = slice(t * 512, (t + 1) * 512)
                            nc.tensor.matmul(
                                po[t], w2h[:, wi, :], mov[:, ts],
                                start=st, stop=sp,
                            )
                    if probe != "noact":
                        ot = outp.tile([128, T], BF16, tag="ot")
                        if fuse:
                            nc.vector.tensor_copy(ot, pob)
                            nc.sync.dma_start(out=outT_t[h], in_=ot)
                        elif h == TH - 1:
                            # tail: per-chunk copy+DMA so the final DMA
                            # overlaps the second copy instead of following it
                            for t in range(NT):
                                ts = slice(t * 512, (t + 1) * 512)
                                nc.vector.tensor_copy(ot[:, ts], po[t])
                                nc.sync.dma_start(
                                    out=outT_t[h][:, ts], in_=ot[:, ts]
                                )
                        else:
                            for t in range(NT):
                                ts = slice(t * 512, (t + 1) * 512)
                                nc.vector.tensor_copy(ot[:, ts], po[t])
                            nc.sync.dma_start(out=outT_t[h], in_=ot)

    if postpasses:
        if dedup:
            _dedup_ldweights(nc)
        if coalesce:
            _coalesce_pe_updates(nc)
        _split_waits(nc, 1)
    return nc


def make_in_maps(hidden_states, gate_up_proj, down_proj):
    import ml_dtypes

    BF = ml_dtypes.bfloat16
    x = np.asarray(hidden_states, dtype=np.float32).reshape(NUM_EXPERTS, T, HIDDEN)
    w1 = np.asarray(gate_up_proj, dtype=np.float32)
    w2 = np.asarray(down_proj, dtype=np.float32)
    in_maps = []
    for e in range(NUM_EXPERTS):
        # (H, 2I) -> (2I/128 g, 128 p, H/128 kk, 128 f) tile-major contiguous
        w1r = w1[e].reshape(TK, 128, 2 * TI, 128).transpose(2, 1, 0, 3)
        # (I, H) -> (H/128 h, 128 p, I/128 ii, 128 f)
        w2r = w2[e].reshape(TI, 128, TH, 128).transpose(2, 1, 0, 3)
        in_maps.append(
            {
                "xT": np.ascontiguousarray(x[e].T).astype(BF),
                "w1": np.ascontiguousarray(w1r).astype(BF),
                "w2": np.ascontiguousarray(w2r).astype(BF),
            }
        )
    return in_maps


def assemble_output(results):
    outs = [results[e]["outT"].T.astype(np.float32) for e in range(NUM_EXPERTS)]
    return np.concatenate(outs, axis=0)


def kernel(hidden_states, gate_up_proj, down_proj):
    from concourse.bass_utils import run_bass_kernel_spmd

    nc = build_bass()
    in_maps = make_in_maps(hidden_states, gate_up_proj, down_proj)
    res = run_bass_kernel_spmd(nc, in_maps, list(range(NUM_EXPERTS)))
    return assemble_output(res.results)



# revision 46
# speedup vs baseline: 1.2255x; 1.2255x over previous
"""Llama4TextExperts MoE kernel for 8 Trainium2 NeuronCores — v4 (bf16).

Expert-parallel: core e handles expert e (tokens pre-sorted per expert).
Per core: x_e (1024,2048) @ gate_up[e] (2048,8192) -> silu(gate)*up ->
@ down[e] (4096,2048) -> out_e (1024,2048).

v2 (bf16): all matmuls bf16 (rel err ~4e-3 vs 2e-2 gate); halves weight
DMA vs fp32r and keeps acted SBUF-resident; LDW dedup post-pass (one
LDWEIGHTS per weight tile, serving both 512-token chunks).

v4 on top of v2 (measured HW findings that motivated it):
- HW probes showed the kernel is matmul-stream-bound: removing all
  LDWEIGHTS + weight DMA (shared-weight probe) did not change exec time,
  so LDW is fully hidden and DMA is not on the critical path. Sustained
  per-MM rate ~270 ns at N=512 (PE ~1.95-2.0 GHz power state); body time
  via repeat-8/repeat-4 NEFFs = ~830 us vs 807 us pure-MM -> ~22 us of
  non-MM slack, i.e. near the sustained-power roofline for bf16.
- bf16 output (+ host upcast) and bf16 silu temp: halves output DMA and
  silu SBUF traffic; rel err 6.3e-3, still 3x under the gate.
- Startup/transition trims (v6, CoreSim-gap-driven): the startup window
  is HBM-BW-saturated, so issue order is consumption order — x[0], the
  first two kk-slices of the g=0 weights (first matmul at ~0.6us),
  x[1..3], weight remainder, x[4..15]; the 2 MB w2 phase-2 prefetch is
  issued mid-phase-1 (g==6) where the DMA queue has slack. Last h's
  output is copy+DMA'd per 512-chunk so the final DMA overlaps the
  second copy. Model: 677.5us (v2) -> 664.2us, PE gaps 21us -> ~9us.
- Sem-update coalescing post-pass: 3072 per-MM +1 posts -> ~160 waited
  checkpoints (accumulated base rides an in-order PE NoOp; the +1 at
  each checkpoint MM still fires at its completion, so no wait can be
  satisfied early). Measured neutral on HW but reduces NX sem traffic.
- fuse=True (wide [128,1024] PSUM tiles, single wide act/mul) compiles
  but hangs the device: Act/DVE APs must not cross a PSUM bank boundary,
  and DVE TensorTensor may read at most one PSUM operand. Kept off.
"""

import numpy as np

NUM_EXPERTS = 8
HIDDEN = 2048
INTER = 4096
TOKENS = 8192
T = TOKENS // NUM_EXPERTS  # 1024 tokens per expert/core
TK = HIDDEN // 128  # 16 contraction tiles in phase 1
TI = INTER // 128  # 32 feature tiles of gate/up; contraction tiles in phase 2
TH = HIDDEN // 128  # 16 output feature tiles
NT = T // 512  # 2 token chunks of 512


def _split_waits(nc, max_waits=1):
    """The walrus build in this environment rejects instructions carrying
    more than one sync wait. Move excess SyncWaits onto preceding NoOps
    on the same engine."""
    import concourse.mybir as mybir

    for fn in nc.m.functions:
        for blk in fn.blocks:
            new_insts = []
            for inst in blk.instructions:
                si = inst.sync_info
                if si is not None and len(si.on_wait) > max_waits:
                    waits = list(si.on_wait)
                    excess, keep = waits[:-max_waits], waits[-max_waits:]
                    for i in range(0, len(excess), max_waits):
                        chunk = excess[i : i + max_waits]
                        new_insts.append(
                            mybir.InstNoOp(
                                name=f"{inst.name}-waitsplit-{i}",
                                ins=[],
                                outs=[],
                                engine=inst.engine,
                                sync_info=mybir.SyncInfo(
                                    on_wait=list(chunk), on_update=[]
                                ),
                            )
                        )
                    si.on_wait = keep
                new_insts.append(inst)
            blk.instructions = new_insts


def _dedup_ldweights(nc):
    """Remove InstLdweights whose weights AP equals the one already loaded
    (Tile emits 1:1 LDW:MM). Waits/updates of a removed LDW attach to the
    next instruction: both then take effect later than the original LDW
    would have, which is always safe."""
    import concourse.mybir as mybir

    removed = 0
    for fn in nc.m.functions:
        for blk in fn.blocks:
            loaded = None
            pending = []
            final = []
            for inst in blk.instructions:
                if isinstance(inst, mybir.InstLdweights):
                    key = repr(inst.ins[0])
                    if loaded == key and inst.perf_mode is None:
                        si = inst.sync_info
                        if si is not None:
                            pending.append(
                                (list(si.on_wait), list(si.on_update))
                            )
                        removed += 1
                        continue
                    loaded = key
                elif isinstance(inst, mybir.InstMatmult) and inst.is_transpose:
                    loaded = None
                if pending:
                    si = inst.sync_info
                    if si is None:
                        inst.sync_info = mybir.SyncInfo(on_wait=[], on_update=[])
                        si = inst.sync_info
                    for pw, pu in pending:
                        si.on_wait = list(si.on_wait) + pw
                        si.on_update = list(si.on_update) + pu
                    pending = []
                final.append(inst)
            blk.instructions = final
    return removed


def _coalesce_pe_updates(nc):
    """Each matmul posts +1 to the PE counting semaphore, but only ~160
    counts are ever waited on (group completions). Remove the per-MM posts
    and emit one accumulated sem-add at each waited count instead. Waits are
    sem-ge-imm (monotonic), so posting the same totals later-but-batched
    preserves all orderings; counts at every waited threshold are exact."""
    import concourse.mybir as mybir

    # identify the PE counting sem: the one inc'd by matmuls
    from collections import Counter

    upd_count = Counter()
    for fn in nc.m.functions:
        for blk in fn.blocks:
            for inst in blk.instructions:
                if isinstance(inst, mybir.InstMatmult) and inst.sync_info:
                    for u in inst.sync_info.on_update:
                        if u.update_mode == "sem-inc" and u.update_value == 1:
                            upd_count[u.id] += 1
    if not upd_count:
        return 0
    sem_id, n = upd_count.most_common(1)[0]
    if n < 100:
        return 0

    thresholds = set()
    for fn in nc.m.functions:
        for blk in fn.blocks:
            for inst in blk.instructions:
                if inst.sync_info:
                    for w in inst.sync_info.on_wait:
                        if w.id == sem_id:
                            thresholds.add(w.wait_value)

    def catchup_noop(name, pending):
        # Generic PE-queue instruction carrying the accumulated base add.
        # NoOps execute in order (only LDWEIGHTS gets pulled ahead), and no
        # wait threshold falls inside the covered range, so posting the base
        # early-at-dispatch is safe: waited counts are only reached by the
        # +1 incs that fire at the checkpoint MMs' own completion.
        return mybir.InstNoOp(
            name=name,
            ins=[],
            outs=[],
            engine=mybir.EngineType.PE,
            sync_info=mybir.SyncInfo(
                on_wait=[],
                on_update=[
                    mybir.SyncUpdate(
                        sync_type="semaphore",
                        id=sem_id,
                        update_mode="sem-add-imm",
                        update_value=pending,
                    )
                ],
            ),
        )

    removed = 0
    for fn in nc.m.functions:
        for blk in fn.blocks:
            count = 0
            pending = 0
            new_insts = []
            for inst in blk.instructions:
                si = inst.sync_info
                ours = (
                    [u for u in si.on_update if u.id == sem_id] if si else []
                )
                if (
                    ours
                    and isinstance(inst, mybir.InstMatmult)
                    and len(ours) == 1
                    and ours[0].update_mode == "sem-inc"
                    and ours[0].update_value == 1
                ):
                    count += 1
                    pending += 1
                    if count in thresholds:
                        if pending > 1:
                            new_insts.append(
                                catchup_noop(f"{inst.name}-semcatch", pending - 1)
                            )
                            removed += pending - 1
                        pending = 0
                        # keep the +1 inc on this MM
                    else:
                        si.on_update = [
                            x for x in si.on_update if x is not ours[0]
                        ]
                elif ours:
                    # foreign update on this sem: post pending base first
                    if pending:
                        new_insts.append(
                            catchup_noop(f"{inst.name}-semflush", pending)
                        )
                        pending = 0
                new_insts.append(inst)
            if pending:
                new_insts.append(catchup_noop(f"{blk.name}-semtail", pending))
            blk.instructions = new_insts
    return removed


def build_bass(
    repeat=1, postpasses=True, probe=None, dedup=True, coalesce=True, fuse=False
):
    # fuse=True (wide [128,1024] PSUM tiles + single wide act/mul/copy per
    # block) compiles but hangs the device — Act/DVE reads crossing a PSUM
    # bank boundary appear to be illegal on HW. Keep off.
    import contextlib

    import concourse.bass as bass
    import concourse.mybir as mybir
    import concourse.tile as tile

    F32 = mybir.dt.float32
    BF16 = mybir.dt.bfloat16
    Silu = mybir.ActivationFunctionType.Silu

    nc = bass.Bass()
    xT = nc.declare_dram_parameter("xT", [HIDDEN, T], BF16, isOutput=False)
    # host-reordered tile-major: w1[g, p, kk, f] = gate_up[kk*128+p, g*128+f]
    # (g 0..31 = gate blocks, 32..63 = up blocks); w2[h, p, ii, f] = down[ii*128+p, h*128+f]
    w1 = nc.declare_dram_parameter("w1", [2 * TI, 128, TK, 128], BF16, isOutput=False)
    w2 = nc.declare_dram_parameter("w2", [TH, 128, TI, 128], BF16, isOutput=False)
    # bf16 output (host upcasts): halves the output DMA + copy traffic; adds
    # <=0.4% rounding on top of ~0.44% — still well under the 2e-2 gate
    outT = nc.declare_dram_parameter("outT", [HIDDEN, T], BF16, isOutput=True)

    xT_t = xT.rearrange("(kk p) t -> kk p t", p=128)
    outT_t = outT.rearrange("(hh p) t -> hh p t", p=128)

    with tile.TileContext(nc) as tc:
        rep = tc.For_i(0, repeat, 1) if repeat > 1 else contextlib.nullcontext()
        with rep:
            with tc.tile_pool(name="xres", bufs=1) as xp, \
                 tc.tile_pool(name="w1s", bufs=3) as w1p, \
                 tc.tile_pool(name="tmp", bufs=8) as tmpp, \
                 tc.tile_pool(name="acted", bufs=1) as actp, \
                 tc.tile_pool(name="w2s", bufs=3) as w2p, \
                 tc.tile_pool(name="outs", bufs=4) as outp, \
                 tc.tile_pool(name="ps", bufs=2, space="PSUM") as psp:
                xts = [
                    xp.tile([128, T], BF16, tag=f"x{k}", name=f"xres{k}")
                    for k in range(TK)
                ]
                # DMA issue order tuned for startup latency: first 4 x tiles,
                # then the g=0 gate/up weights, then the remaining x. The
                # startup window is HBM-BW-saturated, so the 2 MB phase-2 w2
                # prefetch is issued mid-phase-1 (g==6) instead — it only has
                # to land ~650us later.
                wg0 = wu0 = w2t0 = w2t1 = None
                nc.sync.dma_start(out=xts[0], in_=xT_t[0])
                if probe not in ("noldw", "mmonly"):
                    wg0 = w1p.tile([128, TK, 128], BF16, tag="wg")
                    wu0 = w1p.tile([128, TK, 128], BF16, tag="wu")
                    nc.sync.dma_start(out=wg0[:, 0:2, :], in_=w1[0, :, 0:2, :])
                    nc.sync.dma_start(out=wu0[:, 0:2, :], in_=w1[TI, :, 0:2, :])
                for k in range(1, 4):
                    nc.sync.dma_start(out=xts[k], in_=xT_t[k])
                if probe not in ("noldw", "mmonly"):
                    nc.sync.dma_start(out=wg0[:, 2:, :], in_=w1[0, :, 2:, :])
                    nc.sync.dma_start(out=wu0[:, 2:, :], in_=w1[TI, :, 2:, :])
                    w2t0 = w2p.tile([128, TI, 128], BF16, tag="w2")
                    w2t1 = w2p.tile([128, TI, 128], BF16, tag="w2")
                for k in range(4, TK):
                    nc.sync.dma_start(out=xts[k], in_=xT_t[k])

                actts = (
                    [
                        actp.tile([128, T], BF16, tag=f"a{i}", name=f"acted{i}")
                        for i in range(TI)
                    ]
                    if probe not in ("noact", "mmonly")
                    else None
                )

                # probe="mmonly": flat stream of 3072 N=512 matmuls, shared
                # weights, 2 psum banks, two long accumulation groups — no
                # consumers, no bank cycling. Isolates the raw MM issue rate.
                if probe == "mmonly":
                    w_sh = w1p.tile([128, TK, 128], BF16, tag="wg")
                    nc.sync.dma_start(out=w_sh, in_=w1[0])
                    ps = [
                        psp.tile([128, 512], F32, tag=f"pg{t}", name=f"psmm{t}")
                        for t in range(2)
                    ]
                    NMM = 3072
                    for i in range(NMM):
                        b = i % 2
                        nc.tensor.matmul(
                            ps[b],
                            w_sh[:, 0, :],
                            xts[i % TK][:, (i // TK % NT) * 512:(i // TK % NT) * 512 + 512],
                            start=(i < 2),
                            stop=(i >= NMM - 2),
                        )
                    ot = outp.tile([128, T], BF16, tag="ot")
                    for t in range(NT):
                        nc.vector.tensor_copy(ot[:, t * 512:(t + 1) * 512], ps[t])
                    nc.sync.dma_start(out=outT_t[0], in_=ot)

                # probe="noldw": single shared weight tile for every matmul so
                # the dedup pass strips all but ~1 LDWEIGHTS — isolates the
                # serialized LDW cost on HW (output is garbage; timing only).
                if probe == "noldw":
                    wg_sh = w1p.tile([128, TK, 128], BF16, tag="wg")
                    wu_sh = wg_sh
                    nc.sync.dma_start(out=wg_sh, in_=w1[0])

                # ---- Phase 1: gate/up projection + silu(gate)*up -> acted ----
                for g in range(TI if probe != "mmonly" else 0):
                    if probe == "noldw":
                        wg, wu = wg_sh, wu_sh
                    elif g == 0:
                        wg, wu = wg0, wu0
                    else:
                        wg = w1p.tile([128, TK, 128], BF16, tag="wg")
                        wu = w1p.tile([128, TK, 128], BF16, tag="wu")
                        nc.sync.dma_start(out=wg, in_=w1[g])
                        nc.sync.dma_start(out=wu, in_=w1[TI + g])
                    if g == 6 and probe not in ("noldw", "mmonly"):
                        nc.sync.dma_start(out=w2t0, in_=w2[0])
                        nc.sync.dma_start(out=w2t1, in_=w2[1])
                    if fuse:
                        # [128,1024] psum tiles span 2 banks; each matmul
                        # still writes a single bank. silu is computed
                        # in-place in psum (no SBUF temp), then one wide DVE
                        # mul produces the bf16 acted tile.
                        pgb = psp.tile([128, T], F32, tag="pg", name=f"psg{g}")
                        pub = psp.tile([128, T], F32, tag="pu", name=f"psu{g}")
                        pg = [pgb[:, t * 512:(t + 1) * 512] for t in range(NT)]
                        pu = [pub[:, t * 512:(t + 1) * 512] for t in range(NT)]
                    else:
                        pg = [
                            psp.tile([128, 512], F32, tag=f"pg{t}", name=f"psg{g}_{t}")
                            for t in range(NT)
                        ]
                        pu = [
                            psp.tile([128, 512], F32, tag=f"pu{t}", name=f"psu{g}_{t}")
                            for t in range(NT)
                        ]
                    for kk in range(TK):
                        st, sp = kk == 0, kk == TK - 1
                        wk = 0 if probe == "noldw" else kk
                        for t in range(NT):
                            ts = slice(t * 512, (t + 1) * 512)
                            nc.tensor.matmul(
                                pg[t], wg[:, wk, :], xts[kk][:, ts], start=st, stop=sp
                            )
                        for t in range(NT):
                            ts = slice(t * 512, (t + 1) * 512)
                            nc.tensor.matmul(
                                pu[t], wu[:, wk, :], xts[kk][:, ts], start=st, stop=sp
                            )
                    if probe != "noact":
                        if fuse:
                            # DVE can read at most one PSUM operand: silu
                            # goes to an SBUF bf16 temp, one wide instr each
                            sgw = tmpp.tile([128, T], BF16, tag="sg")
                            nc.scalar.activation(sgw, pgb, Silu)
                            nc.vector.tensor_mul(actts[g], sgw, pub)
                        else:
                            for t in range(NT):
                                ts = slice(t * 512, (t + 1) * 512)
                                sg = tmpp.tile([128, 512], BF16, tag="sg")
                                nc.scalar.activation(sg, pg[t], Silu)
                                nc.vector.tensor_mul(actts[g][:, ts], sg, pu[t])

                # ---- Phase 2: down projection -> outT ----
                if probe == "noldw":
                    w2_sh = w2p.tile([128, TI, 128], BF16, tag="w2")
                    nc.sync.dma_start(out=w2_sh, in_=w2[0])
                for h in range(TH if probe != "mmonly" else 0):
                    if probe == "noldw":
                        w2h = w2_sh
                    elif h == 0:
                        w2h = w2t0
                    elif h == 1:
                        w2h = w2t1
                    else:
                        w2h = w2p.tile([128, TI, 128], BF16, tag="w2")
                        nc.sync.dma_start(out=w2h, in_=w2[h])
                    if fuse:
                        pob = psp.tile([128, T], F32, tag="pg", name=f"pso{h}")
                        po = [pob[:, t * 512:(t + 1) * 512] for t in range(NT)]
                    else:
                        po = [
                            psp.tile([128, 512], F32, tag=f"pg{t}", name=f"pso{h}_{t}")
                            for t in range(NT)
                        ]
                    for ii in range(TI):
                        st, sp = ii == 0, ii == TI - 1
                        wi = 0 if probe == "noldw" else ii
                        mov = xts[ii % TK] if probe == "noact" else actts[ii]
                        for t in range(NT):
                            ts = slice(t * 512, (t + 1) * 512)
                            nc.tensor.matmul(
                                po[t], w2h[:, wi, :], mov[:, ts],
                                start=st, stop=sp,
                            )
                    if probe != "noact":
                        ot = outp.tile([128, T], BF16, tag="ot")
                        if fuse:
                            nc.vector.tensor_copy(ot, pob)
                            nc.sync.dma_start(out=outT_t[h], in_=ot)
                        elif h == TH - 1:
                            # tail: per-chunk copy+DMA so the final DMA
                            # overlaps the second copy instead of following it
                            for t in range(NT):
                                ts = slice(t * 512, (t + 1) * 512)
                                nc.vector.tensor_copy(ot[:, ts], po[t])
                                nc.sync.dma_start(
                                    out=outT_t[h][:, ts], in_=ot[:, ts]
                                )
                        else:
                            for t in range(NT):
                                ts = slice(t * 512, (t + 1) * 512)
                                nc.vector.tensor_copy(ot[:, ts], po[t])
                            nc.sync.dma_start(out=outT_t[h], in_=ot)

    if postpasses:
        if dedup:
            _dedup_ldweights(nc)
        if coalesce:
            _coalesce_pe_updates(nc)
        _split_waits(nc, 1)
    return nc


def make_in_maps(hidden_states, gate_up_proj, down_proj):
    import ml_dtypes

    BF = ml_dtypes.bfloat16
    x = np.asarray(hidden_states, dtype=np.float32).reshape(NUM_EXPERTS, T, HIDDEN)
    w1 = np.asarray(gate_up_proj, dtype=np.float32)
    w2 = np.asarray(down_proj, dtype=np.float32)
    in_maps = []
    for e in range(NUM_EXPERTS):
        # (H, 2I) -> (2I/128 g, 128 p, H/128 kk, 128 f) tile-major contiguous
        w1r = w1[e].reshape(TK, 128, 2 * TI, 128).transpose(2, 1, 0, 3)
        # (I, H) -> (H/128 h, 128 p, I/128 ii, 128 f)
        w2r = w2[e].reshape(TI, 128, TH, 128).transpose(2, 1, 0, 3)
        in_maps.append(
            {
                "xT": np.ascontiguousarray(x[e].T).astype(BF),
                "w1": np.ascontiguousarray(w1r).astype(BF),
                "w2": np.ascontiguousarray(w2r).astype(BF),
            }
        )
    return in_maps


def assemble_output(results):
    outs = [results[e]["outT"].T.astype(np.float32) for e in range(NUM_EXPERTS)]
    return np.concatenate(outs, axis=0)


def kernel(hidden_states, gate_up_proj, down_proj):
    from concourse.bass_utils import run_bass_kernel_spmd

    nc = build_bass()
    in_maps = make_in_maps(hidden_states, gate_up_proj, down_proj)
    res = run_bass_kernel_spmd(nc, in_maps, list(range(NUM_EXPERTS)))
    return assemble_output(res.results)

